# revision 7
# baseline (speedup 1.0000x reference)
"""CapsuleLayer (dynamic routing) Trainium2 kernel.

Problem: B=128, I=1152 input capsules (A=8), O=10 output capsules (OA=16),
3 routing iterations.  Data-parallel over batch: 8 cores x 16 examples.

Per-core layout ("P2"): SBUF partition p = is*16 + b  (is = i mod 8, b = local
batch), chunk c = i // 8 in the free dim, vote coordinate n = oa*10 + o
(o innermost so the squash/softmax reductions are innermost-axis reductions).

Phases (bf16 data / f32 accumulation):
  votes:  72 paired-chunk matmuls [k=128=(is,h,a), m=128=(is,b), n=320]
          with an on-chip-built block-diagonal x as the stationary operand
          (host ships a compact x; DVE memset + 8 strided copies build the
          block diagonal while DMA streams w2c).  Iter-1 s-matmuls are
          interleaved into the PE queue right behind each PSUM copy-out.
  iter t: softmax(logits) -> wv = votes*route (tapered pieces, overlapping
          the s-matmuls) -> s via 48 accumulating 480-wide matmuls with a
          constant 0/1 selection lhsT (sums over i) -> squash (PSUM-direct
          adds; sqrt via exp/ln so one ACT table set serves the whole
          kernel) -> v replicated to all partitions via a constant matmul ->
          delta = sum_oa votes*v via elementwise mul + in-place pair-tree
          reduce -> logits += delta.  The last iteration stops after squash
          and DMAs v out.
"""

import numpy as np
import ml_dtypes

B, I, A, O, OA = 128, 1152, 8, 10, 16
NCORES = 8
BL = B // NCORES        # 16 examples per core
IS8 = 8                 # i-positions per half-chunk
C = I // IS8            # 144 half-chunks
CP = C // 2             # 72 paired chunks
N = O * OA              # 160, n = oa*O + o
N2 = 2 * N              # 320 per paired chunk
P = 128                 # p = is*BL + b
NUM_ROUTING = 3

_NC_CACHE = {}


def _build_nc():
    from contextlib import ExitStack

    import concourse.tile as tile
    import concourse.mybir as mybir
    from concourse import bacc

    F32 = mybir.dt.float32
    BF16 = mybir.dt.bfloat16
    AF = mybir.ActivationFunctionType
    ALU = mybir.AluOpType
    AX = mybir.AxisListType

    nc = bacc.Bacc()
    xc_d = nc.dram_tensor("xcomp", [P, CP, 2 * BL], BF16, kind="ExternalInput")
    w2c_d = nc.dram_tensor("w2c", [P, CP, N2], BF16, kind="ExternalInput")
    bsel_d = nc.dram_tensor("bsel", [P, BL], BF16, kind="ExternalInput")
    brep_d = nc.dram_tensor("brep", [BL, P], BF16, kind="ExternalInput")
    bias_d = nc.dram_tensor("biasr", [BL, N], F32, kind="ExternalInput")
    vout_d = nc.dram_tensor("vout", [BL, N], F32, kind="ExternalOutput")

    with ExitStack() as ctx:
        tc = ctx.enter_context(tile.TileContext(nc))
        st = ctx.enter_context(tc.tile_pool(name="static", bufs=1))
        itp = ctx.enter_context(tc.tile_pool(name="itp", bufs=1))

        w2c = st.tile([P, CP, N2], BF16)
        votes = st.tile([P, C, N], BF16)
        logits = st.tile([P, C, O], F32)
        bsel = st.tile([P, BL], BF16)
        brep = st.tile([BL, P], BF16)
        biasr = st.tile([BL, N], F32)

        # Prime the (single) activation table set before any real ACT work.
        dum = st.tile([1, 2], F32)
        nc.vector.memset(dum[:, 0:1], 0.0)
        nc.scalar.activation(dum[:, 1:2], dum[:, 0:1], AF.Exp)

        nc.sync.dma_start(out=bsel[:], in_=bsel_d[:])
        nc.sync.dma_start(out=brep[:], in_=brep_d[:])
        nc.sync.dma_start(out=biasr[:], in_=bias_d[:])

        # ---- votes + interleaved iter-1 s-matmuls ----
        GRP = 3     # paired chunks per psum tile (3 banks; x2 bufs = 6 banks)
        SLOT = 512  # psum bank-aligned slot
        NDMA = 8
        SW = 3          # half-chunks per s-matmul
        NS = C // SW    # 48 s-matmuls per iteration
        pss = ctx.enter_context(tc.tile_pool(name="pss", bufs=1, space="PSUM"))

        with tc.tile_pool(name="ph1", bufs=1) as ph1, tc.tile_pool(
            name="psv", bufs=2, space="PSUM"
        ) as psv:
            xcomp = ph1.tile([P, CP, 2 * BL], BF16)
            xbd = ph1.tile([P, CP, P], BF16)
            # block-diagonal x built on-chip: DMA x padded to 32-wide column
            # blocks (DVE partition ranges must be 32-aligned), memset the
            # [128, 72, 128] stationary, copy the four 32x32 diagonal blocks
            nc.sync.dma_start(out=xcomp[0:64], in_=xc_d[0:64])
            nc.sync.dma_start(out=xcomp[64:128], in_=xc_d[64:128])
            nc.vector.memset(xbd[:], 0.0)
            for q in range(4):
                sl = slice(q * 32, (q + 1) * 32)
                nc.vector.tensor_copy(xbd[sl, :, sl], xcomp[sl])
            dstep = CP // NDMA
            for q in range(NDMA):
                sl = slice(q * dstep, (q + 1) * dstep)
                nc.sync.dma_start(out=w2c[:, sl, :], in_=w2c_d[:, sl, :])

            s_ps = pss.tile([BL, SW * N], F32, tag="sps")
            s_idx = 0
            for g in range(CP // GRP):  # 24 groups of 3 pairs = 6 chunks
                ps = psv.tile([P, GRP * SLOT], F32, tag="pv")
                for j in range(GRP):
                    cp = g * GRP + j
                    nc.tensor.matmul(
                        ps[:, j * SLOT : j * SLOT + N2],
                        lhsT=xbd[:, cp, :],
                        rhs=w2c[:, cp, :],
                        start=True,
                        stop=True,
                    )
                src = ps[:].rearrange("p (j s) -> p j s", j=GRP)[:, :, 0:N2]
                dst = votes[:, g * 2 * GRP : (g + 1) * 2 * GRP, :].rearrange(
                    "p (j c2) n -> p j (c2 n)", j=GRP
                )
                if g % 2 == 0:
                    nc.scalar.copy(dst, src)
                else:
                    nc.vector.tensor_copy(dst, src)
                # iter-1 s-matmuls over chunks already copied out
                while s_idx < NS and (s_idx + 1) * SW <= (g + 1) * 2 * GRP:
                    rhs = votes[:, s_idx * SW : (s_idx + 1) * SW, :].rearrange(
                        "p c n -> p (c n)"
                    )
                    nc.tensor.matmul(
                        s_ps[:],
                        lhsT=bsel[:],
                        rhs=rhs,
                        start=(s_idx == 0),
                        stop=(s_idx == NS - 1),
                    )
                    s_idx += 1

        # ---- routing iterations ----
        PIECES = [45, 36, 27, 18, 12, 6]  # tapered wv pieces (each %3==0)

        for t in range(1, NUM_ROUTING + 1):
            if t > 1:
                # softmax + wv fold, pipelined in halves / tapered pieces
                expb = itp.tile([P, C, O], BF16, tag="expb")
                z = itp.tile([P, C], F32, tag="z")
                rz = itp.tile([P, C], F32, tag="rz")
                route = itp.tile([P, C, O], BF16, tag="route")
                wv = itp.tile([P, C, N], BF16, tag="big")
                v4 = votes[:].rearrange("p c (oa o) -> p c oa o", o=O)
                r4 = route[:].unsqueeze(2).broadcast_to([P, C, OA, O])
                w4 = wv[:].rearrange("p c (oa o) -> p c oa o", o=O)
                H = C // 2
                h1, h2 = slice(0, H), slice(H, C)
                nc.scalar.activation(expb[:, h1], logits[:, h1], AF.Exp)
                nc.scalar.activation(expb[:, h2], logits[:, h2], AF.Exp)
                nc.vector.reduce_sum(z[:, h1], expb[:, h1], axis=AX.X)
                nc.vector.reciprocal_approx_fast(rz[:, h1], z[:, h1])
                nc.vector.tensor_mul(
                    route[:, h1],
                    expb[:, h1],
                    rz[:, h1].unsqueeze(2).broadcast_to([P, H, O]),
                )
                s_ps = pss.tile([BL, SW * N], F32, tag="sps")
                s_idx = 0
                done = 0
                for pi, piece in enumerate(PIECES):
                    sl = slice(done, done + piece)
                    nc.vector.tensor_mul(w4[:, sl], v4[:, sl], r4[:, sl])
                    done += piece
                    if pi == 0:
                        # second half of the softmax while PE chews piece 0
                        nc.vector.reduce_sum(z[:, h2], expb[:, h2], axis=AX.X)
                        nc.vector.reciprocal_approx_fast(rz[:, h2], z[:, h2])
                        nc.vector.tensor_mul(
                            route[:, h2],
                            expb[:, h2],
                            rz[:, h2].unsqueeze(2).broadcast_to([P, C - H, O]),
                        )
                    while s_idx < NS and (s_idx + 1) * SW <= done:
                        rhs = wv[:, s_idx * SW : (s_idx + 1) * SW, :].rearrange(
                            "p c n -> p (c n)"
                        )
                        nc.tensor.matmul(
                            s_ps[:],
                            lhsT=bsel[:],
                            rhs=rhs,
                            start=(s_idx == 0),
                            stop=(s_idx == NS - 1),
                        )
                        s_idx += 1

            # s = sum of the 3 chunk-phase partial sums, read from PSUM
            # (one PSUM operand per instruction — PSUM has a single DVE port)
            ps3 = s_ps[:].rearrange("b (c n) -> b c n", c=SW)
            sa = itp.tile([BL, N], F32, tag="sa")
            nc.vector.tensor_copy(sa[:], ps3[:, 0, :])
            nc.vector.tensor_add(sa[:], sa[:], ps3[:, 1, :])
            s_t = itp.tile([BL, N], F32, tag="stile")
            if t == 1:
                # s = (sa + ps3[2]) * (1/O) + bias, folded:
                nc.vector.tensor_add(sa[:], sa[:], ps3[:, 2, :])
                nc.vector.scalar_tensor_tensor(
                    s_t[:], sa[:], 1.0 / O, biasr[:], op0=ALU.mult, op1=ALU.add
                )
            else:
                nc.vector.tensor_add(sa[:], sa[:], ps3[:, 2, :])
                nc.vector.tensor_add(s_t[:], sa[:], biasr[:])

            # squash along o: nsq[b, oa] = sum_o s^2;
            # f = sqrt(nsq)/(1+nsq) = exp(0.5*ln(nsq) - ln(1+nsq))
            sq = itp.tile([BL, N], F32, tag="sq")
            nc.vector.tensor_mul(sq[:], s_t[:], s_t[:])
            nsq = itp.tile([BL, OA], F32, tag="nsq")
            nc.vector.reduce_sum(
                nsq[:], sq[:].rearrange("b (oa o) -> b oa o", o=O), axis=AX.X
            )
            la = itp.tile([BL, OA], F32, tag="la")
            nc.scalar.activation(la[:], nsq[:], AF.Ln)
            lb = itp.tile([BL, OA], F32, tag="lb")
            nc.scalar.activation(lb[:], nsq[:], AF.Ln, bias=1.0)
            lc = itp.tile([BL, OA], F32, tag="lc")
            nc.vector.scalar_tensor_tensor(
                lc[:], la[:], 0.5, lb[:], op0=ALU.mult, op1=ALU.subtract
            )
            f = itp.tile([BL, OA], F32, tag="f")
            nc.scalar.activation(f[:], lc[:], AF.Exp)
            if t == NUM_ROUTING:
                vt = itp.tile([BL, N], F32, tag="vt")
                nc.vector.tensor_mul(
                    vt[:].rearrange("b (oa o) -> b oa o", o=O),
                    s_t[:].rearrange("b (oa o) -> b oa o", o=O),
                    f[:].unsqueeze(2).broadcast_to([BL, OA, O]),
                )
                nc.sync.dma_start(out=vout_d[:], in_=vt[:])
                break

            vtb = itp.tile([BL, N], BF16, tag="vtb")
            nc.vector.tensor_mul(
                vtb[:].rearrange("b (oa o) -> b oa o", o=O),
                s_t[:].rearrange("b (oa o) -> b oa o", o=O),
                f[:].unsqueeze(2).broadcast_to([BL, OA, O]),
            )
            vr_ps = pss.tile([P, N], F32, tag="vrps")
            nc.tensor.matmul(vr_ps[:], lhsT=brep[:], rhs=vtb[:], start=True, stop=True)
            vrep = itp.tile([P, N], BF16, tag="vrep")
            nc.scalar.copy(vrep[:], vr_ps[:])

            tmp = itp.tile([P, C, N], BF16, tag="big")
            vr_b = vrep[:].unsqueeze(1).broadcast_to([P, C, N])
            nc.vector.tensor_mul(tmp[:], votes[:], vr_b[:])
            t4 = tmp[:].rearrange("p c (oa o) -> p c oa o", o=O)
            for h in (8, 4, 2):
                nc.vector.tensor_add(
                    t4[:, :, 0:h, :], t4[:, :, 0:h, :], t4[:, :, h : 2 * h, :]
                )
            H2 = C // 2
            if t == 1:
                for hs in (slice(0, H2), slice(H2, C)):
                    nc.vector.tensor_add(
                        logits[:, hs], t4[:, hs, 0, :], t4[:, hs, 1, :]
                    )
            else:
                # one more (bf16, 2x) tree level, then a single f32 accumulate
                nc.vector.tensor_add(
                    t4[:, :, 0, :], t4[:, :, 0, :], t4[:, :, 1, :]
                )
                nc.vector.tensor_add(logits[:], logits[:], t4[:, :, 0, :])

    nc.compile()
    return nc


def get_nc():
    if "nc" not in _NC_CACHE:
        _NC_CACHE["nc"] = _build_nc()
    return _NC_CACHE["nc"]


def make_in_maps(x, weights, biases):
    bf = ml_dtypes.bfloat16
    x = np.asarray(x, np.float32)
    weights = np.asarray(weights, np.float32)
    biases = np.asarray(biases, np.float32)

    # w2c[(is, h, a), cp, h2*N + (oa, o)] = w[(2cp+h)*8+is, a, o*16+oa] * (h==h2)
    w5 = (
        weights.reshape(CP, 2, IS8, A, O, OA)
        .transpose(0, 1, 2, 3, 5, 4)
        .reshape(CP, 2, IS8, A, N)
    )
    w2c = np.zeros((CP, IS8, 2, A, 2, N), np.float32)
    for h in range(2):
        w2c[:, :, h, :, h, :] = w5[:, h]  # w5[:, h] is [CP, IS8, A, N]
    w2c = w2c.reshape(CP, P, N2).transpose(1, 0, 2).astype(bf)

    eye = np.eye(BL, dtype=np.float32)
    bsel = np.tile(eye, (IS8, 1)).astype(bf)  # bsel[p, b'] = delta(p % BL == b')
    brep = np.tile(eye, (1, IS8)).astype(bf)  # brep[b, p] = delta(b == p % BL)
    biasr = np.broadcast_to(biases.T.reshape(1, N), (BL, N)).astype(np.float32).copy()

    in_maps = []
    for k in range(NCORES):
        xc = x[k * BL : (k + 1) * BL]  # [BL, I, A]
        # xcomp[(is, h, a), cp, (is&1)*16 + b] = x[b, (2cp+h)*8+is, a]
        # (padded to 32-wide column blocks so the on-chip diagonal scatter
        #  can use 32-partition-aligned copies)
        xt = (
            xc.reshape(BL, CP, 2, IS8, A)
            .transpose(3, 2, 4, 1, 0)  # [is, h, a, cp, b]
            .reshape(IS8 // 2, 2, 2 * A, CP, BL)  # [is2, isl, (h,a), cp, b]
        )
        xcomp = np.zeros((IS8 // 2, 2, 2 * A, CP, 2, BL), np.float32)
        for isl in range(2):
            xcomp[:, isl, :, :, isl, :] = xt[:, isl]
        xcomp = xcomp.reshape(P, CP, 2 * BL).astype(bf)
        in_maps.append(
            {
                "xcomp": np.ascontiguousarray(xcomp),
                "w2c": w2c,
                "bsel": bsel,
                "brep": brep,
                "biasr": biasr,
            }
        )
    return in_maps


def assemble_out(results):
    out = np.zeros((B, 1, O, OA), np.float32)
    for k in range(NCORES):
        v = np.asarray(results[k]["vout"], np.float32)  # [BL, N], n = oa*O + o
        out[k * BL : (k + 1) * BL, 0] = v.reshape(BL, OA, O).transpose(0, 2, 1)
    return out


def kernel(x, weights, biases):
    from concourse.bass_utils import run_bass_kernel_spmd

    nc = get_nc()
    in_maps = make_in_maps(x, weights, biases)
    res = run_bass_kernel_spmd(nc, in_maps, list(range(NCORES)))
    return assemble_out(res.results)


# revision 8
# speedup vs baseline: 1.0628x; 1.0628x over previous
"""CapsuleLayer (dynamic routing) Trainium2 kernel.

Problem: B=128, I=1152 input capsules (A=8), O=10 output capsules (OA=16),
3 routing iterations.  Data-parallel over batch: 8 cores x 16 examples.

Per-core layout ("P2"): SBUF partition p = is*16 + b  (is = i mod 8, b = local
batch), chunk c = i // 8 in the free dim, vote coordinate n = oa*10 + o
(o innermost so the squash/softmax reductions are innermost-axis reductions).

Phases (bf16 data / f32 accumulation):
  votes:  72 paired-chunk matmuls [k=128=(is,h,a), m=128=(is,b), n=320]
          with an on-chip-built block-diagonal x as the stationary operand
          (host ships x padded to 32-wide column blocks; a f32-bitcast DVE
          memset + four 32-partition-aligned copies build the block diagonal
          while DMA streams w2c).
  iter t: softmax(logits) -> wv = votes*route (small-first tapered pieces so
          the s-matmuls start early and finish with the fold) -> s via 48
          accumulating 480-wide matmuls with a constant 0/1 selection lhsT
          (sums over i) -> squash (PSUM-direct adds; ACT-table loads for
          exp/sqrt are prefetched into idle scalar windows by 1-element
          dummy activations) -> v replicated to all partitions via a
          constant matmul -> delta = sum_oa votes*v via elementwise mul +
          in-place pair-tree reduce -> logits += delta.  The last iteration
          stops after squash and DMAs v out.
"""

import numpy as np
import ml_dtypes

B, I, A, O, OA = 128, 1152, 8, 10, 16
NCORES = 8
BL = B // NCORES        # 16 examples per core
IS8 = 8                 # i-positions per half-chunk
C = I // IS8            # 144 half-chunks
CP = C // 2             # 72 paired chunks
N = O * OA              # 160, n = oa*O + o
N2 = 2 * N              # 320 per paired chunk
P = 128                 # p = is*BL + b
NUM_ROUTING = 3

_NC_CACHE = {}


def _build_nc():
    from contextlib import ExitStack

    import concourse.tile as tile
    import concourse.mybir as mybir
    from concourse import bacc

    F32 = mybir.dt.float32
    BF16 = mybir.dt.bfloat16
    AF = mybir.ActivationFunctionType
    ALU = mybir.AluOpType
    AX = mybir.AxisListType

    nc = bacc.Bacc()
    xc_d = nc.dram_tensor("xcomp", [P, CP, 2 * BL], BF16, kind="ExternalInput")
    w2c_d = nc.dram_tensor("w2c", [P, CP, N2], BF16, kind="ExternalInput")
    bsel_d = nc.dram_tensor("bsel", [P, BL], BF16, kind="ExternalInput")
    brep_d = nc.dram_tensor("brep", [BL, P], BF16, kind="ExternalInput")
    bias_d = nc.dram_tensor("biasr", [BL, N], F32, kind="ExternalInput")
    vout_d = nc.dram_tensor("vout", [BL, N], F32, kind="ExternalOutput")

    with ExitStack() as ctx:
        tc = ctx.enter_context(tile.TileContext(nc))
        st = ctx.enter_context(tc.tile_pool(name="static", bufs=1))
        itp = ctx.enter_context(tc.tile_pool(name="itp", bufs=1))

        w2c = st.tile([P, CP, N2], BF16)
        votes = st.tile([P, C, N], BF16)
        logits = st.tile([P, C, O], F32)
        bsel = st.tile([P, BL], BF16)
        brep = st.tile([BL, P], BF16)
        biasr = st.tile([BL, N], F32)

        # 1-element dummy activations prefetch ACT table sets into idle
        # scalar windows (sqrt and exp live in different sets; each load
        # costs ~1.3us and would otherwise land on the critical path).
        dum = st.tile([1, 2], F32)
        nc.vector.memset(dum[:, 0:1], 0.0)

        def prefetch(func):
            nc.scalar.activation(dum[:, 1:2], dum[:, 0:1], func)

        prefetch(AF.Sqrt)  # squash-1 comes before the first exp

        nc.sync.dma_start(out=bsel[:], in_=bsel_d[:])
        nc.sync.dma_start(out=brep[:], in_=brep_d[:])
        nc.sync.dma_start(out=biasr[:], in_=bias_d[:])

        # ---- votes ----
        GRP = 4     # paired chunks per psum tile
        SLOT = 512  # psum bank-aligned slot
        NDMA = 8
        with tc.tile_pool(name="ph1", bufs=1) as ph1, tc.tile_pool(
            name="psv", bufs=2, space="PSUM"
        ) as psv:
            xcomp = ph1.tile([P, CP, 2 * BL], BF16)
            xbd = ph1.tile([P, CP, P], BF16)
            # block-diagonal x built on-chip: DMA x padded to 32-wide column
            # blocks (engine partition ranges must be 32-aligned), f32-view
            # memset, then copy the four 32x32 diagonal blocks
            nc.sync.dma_start(out=xcomp[0:64], in_=xc_d[0:64])
            nc.sync.dma_start(out=xcomp[64:128], in_=xc_d[64:128])
            nc.vector.memset(xbd[:].bitcast(F32), 0.0)
            for q in range(4):
                sl = slice(q * 32, (q + 1) * 32)
                nc.vector.tensor_copy(xbd[sl, :, sl], xcomp[sl])
            dstep = CP // NDMA
            for q in range(NDMA):
                sl = slice(q * dstep, (q + 1) * dstep)
                nc.sync.dma_start(out=w2c[:, sl, :], in_=w2c_d[:, sl, :])
            for g in range(CP // GRP):  # 18 groups
                ps = psv.tile([P, GRP * SLOT], F32, tag="pv")
                for j in range(GRP):
                    cp = g * GRP + j
                    nc.tensor.matmul(
                        ps[:, j * SLOT : j * SLOT + N2],
                        lhsT=xbd[:, cp, :],
                        rhs=w2c[:, cp, :],
                        start=True,
                        stop=True,
                    )
                src = ps[:].rearrange("p (j s) -> p j s", j=GRP)[:, :, 0:N2]
                dst = votes[:, g * 2 * GRP : (g + 1) * 2 * GRP, :].rearrange(
                    "p (j c2) n -> p j (c2 n)", j=GRP
                )
                if g % 2 == 0:
                    nc.scalar.copy(dst, src)
                else:
                    nc.vector.tensor_copy(dst, src)

        # ---- routing iterations ----
        pss = ctx.enter_context(tc.tile_pool(name="pss", bufs=1, space="PSUM"))
        SW = 3          # half-chunks per s-matmul
        NS = C // SW    # 48 s-matmuls per iteration
        # small-first taper: PE s-matmuls start almost immediately and track
        # the fold; small last piece keeps the PE tail short too
        PIECES = [9, 30, 45, 36, 18, 6]

        for t in range(1, NUM_ROUTING + 1):
            s_ps = pss.tile([BL, SW * N], F32, tag="sps")
            if t == 1:
                for j in range(NS):
                    rhs = votes[:, j * SW : (j + 1) * SW, :].rearrange(
                        "p c n -> p (c n)"
                    )
                    nc.tensor.matmul(
                        s_ps[:],
                        lhsT=bsel[:],
                        rhs=rhs,
                        start=(j == 0),
                        stop=(j == NS - 1),
                    )
            else:
                # softmax + wv fold, pipelined in halves / tapered pieces
                expb = itp.tile([P, C, O], BF16, tag="expb")
                z = itp.tile([P, C], F32, tag="z")
                rz = itp.tile([P, C], F32, tag="rz")
                route = itp.tile([P, C, O], BF16, tag="route")
                wv = itp.tile([P, C, N], BF16, tag="big")
                v4 = votes[:].rearrange("p c (oa o) -> p c oa o", o=O)
                r4 = route[:].unsqueeze(2).broadcast_to([P, C, OA, O])
                w4 = wv[:].rearrange("p c (oa o) -> p c oa o", o=O)
                H = C // 2
                h1, h2 = slice(0, H), slice(H, C)
                nc.scalar.activation(expb[:, h1], logits[:, h1], AF.Exp)
                nc.scalar.activation(expb[:, h2], logits[:, h2], AF.Exp)
                prefetch(AF.Sqrt)  # hide the sqrt table load under the fold
                nc.vector.reduce_sum(z[:, h1], expb[:, h1], axis=AX.X)
                nc.vector.reciprocal_approx_fast(rz[:, h1], z[:, h1])
                nc.vector.tensor_mul(
                    route[:, h1],
                    expb[:, h1],
                    rz[:, h1].unsqueeze(2).broadcast_to([P, H, O]),
                )
                s_idx = 0
                done = 0
                for pi, piece in enumerate(PIECES):
                    sl = slice(done, done + piece)
                    nc.vector.tensor_mul(w4[:, sl], v4[:, sl], r4[:, sl])
                    done += piece
                    if pi == 0:
                        # second half of the softmax while PE chews piece 0
                        nc.vector.reduce_sum(z[:, h2], expb[:, h2], axis=AX.X)
                        nc.vector.reciprocal_approx_fast(rz[:, h2], z[:, h2])
                        nc.vector.tensor_mul(
                            route[:, h2],
                            expb[:, h2],
                            rz[:, h2].unsqueeze(2).broadcast_to([P, C - H, O]),
                        )
                    while s_idx < NS and (s_idx + 1) * SW <= done:
                        rhs = wv[:, s_idx * SW : (s_idx + 1) * SW, :].rearrange(
                            "p c n -> p (c n)"
                        )
                        nc.tensor.matmul(
                            s_ps[:],
                            lhsT=bsel[:],
                            rhs=rhs,
                            start=(s_idx == 0),
                            stop=(s_idx == NS - 1),
                        )
                        s_idx += 1

            # s = sum of the 3 chunk-phase partial sums, read from PSUM
            # (one PSUM operand per instruction — PSUM has a single DVE port)
            ps3 = s_ps[:].rearrange("b (c n) -> b c n", c=SW)
            sa = itp.tile([BL, N], F32, tag="sa")
            nc.vector.tensor_copy(sa[:], ps3[:, 0, :])
            nc.vector.tensor_add(sa[:], sa[:], ps3[:, 1, :])
            s_t = itp.tile([BL, N], F32, tag="stile")
            if t == 1:
                # s = (sa + ps3[2]) * (1/O) + bias, folded:
                nc.vector.tensor_add(sa[:], sa[:], ps3[:, 2, :])
                nc.vector.scalar_tensor_tensor(
                    s_t[:], sa[:], 1.0 / O, biasr[:], op0=ALU.mult, op1=ALU.add
                )
            else:
                nc.vector.tensor_add(sa[:], sa[:], ps3[:, 2, :])
                nc.vector.tensor_add(s_t[:], sa[:], biasr[:])

            # squash along o: nsq[b, oa] = sum_o s^2
            sq = itp.tile([BL, N], F32, tag="sq")
            nc.vector.tensor_mul(sq[:], s_t[:], s_t[:])
            nsq = itp.tile([BL, OA], F32, tag="nsq")
            nc.vector.reduce_sum(
                nsq[:], sq[:].rearrange("b (oa o) -> b oa o", o=O), axis=AX.X
            )
            nsq1 = itp.tile([BL, OA], F32, tag="nsq1")
            nc.vector.tensor_scalar_add(nsq1[:], nsq[:], 1.0)
            rn1 = itp.tile([BL, OA], F32, tag="rn1")
            nc.vector.reciprocal_approx_fast(rn1[:], nsq1[:])
            sr = itp.tile([BL, OA], F32, tag="sr")
            nc.scalar.activation(sr[:], nsq[:], AF.Sqrt)
            if t < NUM_ROUTING:
                prefetch(AF.Exp)  # hide the exp table load under vmul/tree
            f = itp.tile([BL, OA], F32, tag="f")
            nc.vector.tensor_mul(f[:], sr[:], rn1[:])
            if t == NUM_ROUTING:
                vt = itp.tile([BL, N], F32, tag="vt")
                nc.vector.tensor_mul(
                    vt[:].rearrange("b (oa o) -> b oa o", o=O),
                    s_t[:].rearrange("b (oa o) -> b oa o", o=O),
                    f[:].unsqueeze(2).broadcast_to([BL, OA, O]),
                )
                nc.sync.dma_start(out=vout_d[:], in_=vt[:])
                break

            vtb = itp.tile([BL, N], BF16, tag="vtb")
            nc.vector.tensor_mul(
                vtb[:].rearrange("b (oa o) -> b oa o", o=O),
                s_t[:].rearrange("b (oa o) -> b oa o", o=O),
                f[:].unsqueeze(2).broadcast_to([BL, OA, O]),
            )
            vr_ps = pss.tile([P, N], F32, tag="vrps")
            nc.tensor.matmul(vr_ps[:], lhsT=brep[:], rhs=vtb[:], start=True, stop=True)
            vrep = itp.tile([P, N], BF16, tag="vrep")
            nc.scalar.copy(vrep[:], vr_ps[:])

            tmp = itp.tile([P, C, N], BF16, tag="big")
            vr_b = vrep[:].unsqueeze(1).broadcast_to([P, C, N])
            nc.vector.tensor_mul(tmp[:], votes[:], vr_b[:])
            t4 = tmp[:].rearrange("p c (oa o) -> p c oa o", o=O)
            for h in (8, 4, 2):
                nc.vector.tensor_add(
                    t4[:, :, 0:h, :], t4[:, :, 0:h, :], t4[:, :, h : 2 * h, :]
                )
            H2 = C // 2
            if t == 1:
                for hs in (slice(0, H2), slice(H2, C)):
                    nc.vector.tensor_add(
                        logits[:, hs], t4[:, hs, 0, :], t4[:, hs, 1, :]
                    )
            else:
                # one more (bf16, 2x) tree level, then a single f32 accumulate
                nc.vector.tensor_add(
                    t4[:, :, 0, :], t4[:, :, 0, :], t4[:, :, 1, :]
                )
                nc.vector.tensor_add(logits[:], logits[:], t4[:, :, 0, :])

    nc.compile()
    return nc


def get_nc():
    if "nc" not in _NC_CACHE:
        _NC_CACHE["nc"] = _build_nc()
    return _NC_CACHE["nc"]


def make_in_maps(x, weights, biases):
    bf = ml_dtypes.bfloat16
    x = np.asarray(x, np.float32)
    weights = np.asarray(weights, np.float32)
    biases = np.asarray(biases, np.float32)

    # w2c[(is, h, a), cp, h2*N + (oa, o)] = w[(2cp+h)*8+is, a, o*16+oa] * (h==h2)
    w5 = (
        weights.reshape(CP, 2, IS8, A, O, OA)
        .transpose(0, 1, 2, 3, 5, 4)
        .reshape(CP, 2, IS8, A, N)
    )
    w5 = w5.transpose(0, 2, 1, 3, 4)  # [CP, is, h, a, N]
    w2c = np.zeros((CP, IS8, 2, A, 2, N), np.float32)
    for h in range(2):
        w2c[:, :, h, :, h, :] = w5[:, :, h]
    w2c = w2c.reshape(CP, P, N2).transpose(1, 0, 2).astype(bf)

    eye = np.eye(BL, dtype=np.float32)
    bsel = np.tile(eye, (IS8, 1)).astype(bf)  # bsel[p, b'] = delta(p % BL == b')
    brep = np.tile(eye, (1, IS8)).astype(bf)  # brep[b, p] = delta(b == p % BL)
    biasr = np.broadcast_to(biases.T.reshape(1, N), (BL, N)).astype(np.float32).copy()

    in_maps = []
    for k in range(NCORES):
        xc = x[k * BL : (k + 1) * BL]  # [BL, I, A]
        # xcomp[(is, h, a), cp, (is&1)*16 + b] = x[b, (2cp+h)*8+is, a]
        # (padded to 32-wide column blocks so the on-chip diagonal scatter
        #  can use 32-partition-aligned copies)
        xt = (
            xc.reshape(BL, CP, 2, IS8, A)
            .transpose(3, 2, 4, 1, 0)  # [is, h, a, cp, b]
            .reshape(IS8 // 2, 2, 2 * A, CP, BL)  # [is2, isl, (h,a), cp, b]
        )
        xcomp = np.zeros((IS8 // 2, 2, 2 * A, CP, 2, BL), np.float32)
        for isl in range(2):
            xcomp[:, isl, :, :, isl, :] = xt[:, isl]
        xcomp = xcomp.reshape(P, CP, 2 * BL).astype(bf)
        in_maps.append(
            {
                "xcomp": np.ascontiguousarray(xcomp),
                "w2c": w2c,
                "bsel": bsel,
                "brep": brep,
                "biasr": biasr,
            }
        )
    return in_maps


def assemble_out(results):
    out = np.zeros((B, 1, O, OA), np.float32)
    for k in range(NCORES):
        v = np.asarray(results[k]["vout"], np.float32)  # [BL, N], n = oa*O + o
        out[k * BL : (k + 1) * BL, 0] = v.reshape(BL, OA, O).transpose(0, 2, 1)
    return out


def kernel(x, weights, biases):
    from concourse.bass_utils import run_bass_kernel_spmd

    nc = get_nc()
    in_maps = make_in_maps(x, weights, biases)
    res = run_bass_kernel_spmd(nc, in_maps, list(range(NCORES)))
    return assemble_out(res.results)


# revision 11
# speedup vs baseline: 1.1449x; 1.0772x over previous
"""CapsuleLayer (dynamic routing) Trainium2 kernel.

Problem: B=128, I=1152 input capsules (A=8), O=10 output capsules (OA=16),
3 routing iterations.  Data-parallel over batch: 8 cores x 16 examples.

Per-core layout ("P2"): SBUF partition p = is*16 + b  (is = i mod 8, b = local
batch), chunk c = i // 8 in the free dim, vote coordinate n = oa*10 + o
(o innermost so the squash/softmax reductions are innermost-axis reductions).

Phases (bf16 data / f32 accumulation):
  votes:  72 paired-chunk matmuls [k=128=(is,h,a), m=128=(is,b), n=320]
          with an on-chip-built block-diagonal x as the stationary operand
          (host ships x padded to 32-wide column blocks; a f32-bitcast DVE
          memset + four 32-partition-aligned copies build the block diagonal
          while DMA streams w2c).
  iter t: softmax(logits) -> wv = votes*route (small-first tapered pieces so
          the s-matmuls start early and finish with the fold) -> s via 48
          accumulating 480-wide matmuls with a constant 0/1 selection lhsT
          (sums over i) -> squash (PSUM-direct adds; ACT-table loads for
          exp/sqrt are prefetched into idle scalar windows by 1-element
          dummy activations) -> v replicated to all partitions via a
          constant matmul -> delta = sum_oa votes*v via elementwise mul +
          in-place pair-tree reduce -> logits += delta.  The last iteration
          stops after squash and DMAs v out.
"""

import numpy as np
import ml_dtypes

B, I, A, O, OA = 128, 1152, 8, 10, 16
NCORES = 8
BL = B // NCORES        # 16 examples per core
IS8 = 8                 # i-positions per half-chunk
C = I // IS8            # 144 half-chunks
CP = C // 2             # 72 paired chunks
N = O * OA              # 160, n = oa*O + o
N2 = 2 * N              # 320 per paired chunk
P = 128                 # p = is*BL + b
NUM_ROUTING = 3

_NC_CACHE = {}


def _build_nc():
    from contextlib import ExitStack

    import concourse.tile as tile
    import concourse.mybir as mybir
    from concourse import bacc

    F32 = mybir.dt.float32
    BF16 = mybir.dt.bfloat16
    AF = mybir.ActivationFunctionType
    ALU = mybir.AluOpType
    AX = mybir.AxisListType

    nc = bacc.Bacc()
    xc_d = nc.dram_tensor("xcomp", [P, CP, 2 * BL], BF16, kind="ExternalInput")
    w2c_d = nc.dram_tensor("w2c", [P, CP, N2], BF16, kind="ExternalInput")
    bsel_d = nc.dram_tensor("bsel", [P, BL], BF16, kind="ExternalInput")
    brep_d = nc.dram_tensor("brep", [BL, P], BF16, kind="ExternalInput")
    bias_d = nc.dram_tensor("biasr", [BL, N], F32, kind="ExternalInput")
    vout_d = nc.dram_tensor("vout", [BL, N], F32, kind="ExternalOutput")

    with ExitStack() as ctx:
        tc = ctx.enter_context(tile.TileContext(nc))
        st = ctx.enter_context(tc.tile_pool(name="static", bufs=1))
        itp = ctx.enter_context(tc.tile_pool(name="itp", bufs=1))

        w2c = st.tile([P, CP, N2], BF16)
        votes = st.tile([P, C, N], BF16)
        logits = st.tile([P, C, O], F32)
        bsel = st.tile([P, BL], BF16)
        brep = st.tile([BL, P], BF16)
        biasr = st.tile([BL, N], F32)

        # 1-element dummy activations prefetch ACT table sets into idle
        # scalar windows (sqrt and exp live in different sets; each load
        # costs ~1.3us and would otherwise land on the critical path).
        dum = st.tile([1, 2], F32)
        nc.vector.memset(dum[:, 0:1], 0.0)

        def prefetch(func):
            nc.scalar.activation(dum[:, 1:2], dum[:, 0:1], func)

        # ---- votes ----
        GRP = 3     # paired chunks per psum tile (3 banks x2 bufs = 6 banks,
        SLOT = 512  # leaving 2 banks for the s/vrep accumulators)
        SW = 3          # half-chunks per s-matmul
        NS = C // SW    # 48 s-matmuls per iteration
        pss = ctx.enter_context(tc.tile_pool(name="pss", bufs=1, space="PSUM"))
        # reserve the pss banks before psv claims the rest, so the iter-1
        # s-matmuls don't serialize on the phase-1 pool-close barrier
        s_ps = pss.tile([BL, SW * N], F32, tag="sps")
        vr_ps = pss.tile([P, N], F32, tag="vrps")

        with tc.tile_pool(name="ph1", bufs=1) as ph1, tc.tile_pool(
            name="psv", bufs=2, space="PSUM"
        ) as psv:
            xcomp = ph1.tile([P, CP, 2 * BL], BF16)
            xbd = ph1.tile([P, CP, P], BF16)
            # block-diagonal x built on-chip: DMA x padded to 32-wide column
            # blocks (engine partition ranges must be 32-aligned), f32-view
            # memset, then copy the four 32x32 diagonal blocks
            nc.sync.dma_start(out=xcomp[0:64], in_=xc_d[0:64])
            nc.sync.dma_start(out=xcomp[64:128], in_=xc_d[64:128])
            nc.vector.memset(xbd[:].bitcast(F32), 0.0)
            for q in range(4):
                sl = slice(q * 32, (q + 1) * 32)
                nc.vector.tensor_copy(xbd[sl, :, sl], xcomp[sl])
            # small first w2c slice so the first matmul starts early; the
            # small constant tensors ride after it
            nc.sync.dma_start(out=w2c[:, 0:3, :], in_=w2c_d[:, 0:3, :])
            nc.sync.dma_start(out=bsel[:], in_=bsel_d[:])
            nc.sync.dma_start(out=brep[:], in_=brep_d[:])
            nc.sync.dma_start(out=biasr[:], in_=bias_d[:])
            for lo, hi in ((3, 12), (12, 21), (21, 30), (30, 39), (39, 48),
                           (48, 57), (57, 66), (66, 72)):
                nc.sync.dma_start(out=w2c[:, lo:hi, :], in_=w2c_d[:, lo:hi, :])
            for g in range(CP // GRP):  # 24 groups
                ps = psv.tile([P, GRP * SLOT], F32, tag="pv")
                for j in range(GRP):
                    cp = g * GRP + j
                    nc.tensor.matmul(
                        ps[:, j * SLOT : j * SLOT + N2],
                        lhsT=xbd[:, cp, :],
                        rhs=w2c[:, cp, :],
                        start=True,
                        stop=True,
                    )
                src = ps[:].rearrange("p (j s) -> p j s", j=GRP)[:, :, 0:N2]
                dst = votes[:, g * 2 * GRP : (g + 1) * 2 * GRP, :].rearrange(
                    "p (j c2) n -> p j (c2 n)", j=GRP
                )
                if g % 2 == 0:
                    nc.scalar.copy(dst, src)
                else:
                    nc.vector.tensor_copy(dst, src)
            # hide the sqrt table load (needed by squash-1) under the
            # iter-1 s-matmul window
            prefetch(AF.Sqrt)

        # ---- routing iterations ----
        # small-first taper: PE s-matmuls start almost immediately and track
        # the fold; small last piece keeps the PE tail short too
        PIECES = [9, 30, 45, 36, 18, 6]

        for t in range(1, NUM_ROUTING + 1):
            s_ps = pss.tile([BL, SW * N], F32, tag="sps")
            if t == 1:
                for j in range(NS):
                    rhs = votes[:, j * SW : (j + 1) * SW, :].rearrange(
                        "p c n -> p (c n)"
                    )
                    nc.tensor.matmul(
                        s_ps[:],
                        lhsT=bsel[:],
                        rhs=rhs,
                        start=(j == 0),
                        stop=(j == NS - 1),
                    )
            else:
                # softmax + wv fold, pipelined in halves / tapered pieces
                expb = itp.tile([P, C, O], BF16, tag="expb")
                z = itp.tile([P, C], F32, tag="z")
                rz = itp.tile([P, C], F32, tag="rz")
                route = itp.tile([P, C, O], BF16, tag="route")
                wv = itp.tile([P, C, N], BF16, tag="big")
                v4 = votes[:].rearrange("p c (oa o) -> p c oa o", o=O)
                r4 = route[:].unsqueeze(2).broadcast_to([P, C, OA, O])
                w4 = wv[:].rearrange("p c (oa o) -> p c oa o", o=O)
                H = C // 2
                h1, h2 = slice(0, H), slice(H, C)
                nc.scalar.activation(expb[:, h1], logits[:, h1], AF.Exp)
                nc.scalar.activation(expb[:, h2], logits[:, h2], AF.Exp)
                prefetch(AF.Sqrt)  # hide the sqrt table load under the fold
                nc.vector.reduce_sum(z[:, h1], expb[:, h1], axis=AX.X)
                nc.vector.reciprocal_approx_fast(rz[:, h1], z[:, h1])
                nc.vector.tensor_mul(
                    route[:, h1],
                    expb[:, h1],
                    rz[:, h1].unsqueeze(2).broadcast_to([P, H, O]),
                )
                s_idx = 0
                done = 0
                for pi, piece in enumerate(PIECES):
                    sl = slice(done, done + piece)
                    nc.vector.tensor_mul(w4[:, sl], v4[:, sl], r4[:, sl])
                    done += piece
                    if pi == 0:
                        # second half of the softmax while PE chews piece 0
                        nc.vector.reduce_sum(z[:, h2], expb[:, h2], axis=AX.X)
                        nc.vector.reciprocal_approx_fast(rz[:, h2], z[:, h2])
                        nc.vector.tensor_mul(
                            route[:, h2],
                            expb[:, h2],
                            rz[:, h2].unsqueeze(2).broadcast_to([P, C - H, O]),
                        )
                    while s_idx < NS and (s_idx + 1) * SW <= done:
                        rhs = wv[:, s_idx * SW : (s_idx + 1) * SW, :].rearrange(
                            "p c n -> p (c n)"
                        )
                        nc.tensor.matmul(
                            s_ps[:],
                            lhsT=bsel[:],
                            rhs=rhs,
                            start=(s_idx == 0),
                            stop=(s_idx == NS - 1),
                        )
                        s_idx += 1

            # s = sum of the 3 chunk-phase partial sums, read from PSUM
            # (one PSUM operand per instruction — PSUM has a single DVE port)
            ps3 = s_ps[:].rearrange("b (c n) -> b c n", c=SW)
            sa = itp.tile([BL, N], F32, tag="sa")
            nc.vector.tensor_copy(sa[:], ps3[:, 0, :])
            nc.vector.tensor_add(sa[:], sa[:], ps3[:, 1, :])
            s_t = itp.tile([BL, N], F32, tag="stile")
            if t == 1:
                # s = (sa + ps3[2]) * (1/O) + bias, folded:
                nc.vector.tensor_add(sa[:], sa[:], ps3[:, 2, :])
                nc.vector.scalar_tensor_tensor(
                    s_t[:], sa[:], 1.0 / O, biasr[:], op0=ALU.mult, op1=ALU.add
                )
            else:
                nc.vector.tensor_add(sa[:], sa[:], ps3[:, 2, :])
                nc.vector.tensor_add(s_t[:], sa[:], biasr[:])

            # squash along o: nsq[b, oa] = sum_o s^2
            sq = itp.tile([BL, N], F32, tag="sq")
            nc.vector.tensor_mul(sq[:], s_t[:], s_t[:])
            nsq = itp.tile([BL, OA], F32, tag="nsq")
            nc.vector.reduce_sum(
                nsq[:], sq[:].rearrange("b (oa o) -> b oa o", o=O), axis=AX.X
            )
            nsq1 = itp.tile([BL, OA], F32, tag="nsq1")
            nc.vector.tensor_scalar_add(nsq1[:], nsq[:], 1.0)
            rn1 = itp.tile([BL, OA], F32, tag="rn1")
            nc.vector.reciprocal_approx_fast(rn1[:], nsq1[:])
            sr = itp.tile([BL, OA], F32, tag="sr")
            nc.scalar.activation(sr[:], nsq[:], AF.Sqrt)
            if t < NUM_ROUTING:
                prefetch(AF.Exp)  # hide the exp table load under vmul/tree
            f = itp.tile([BL, OA], F32, tag="f")
            nc.vector.tensor_mul(f[:], sr[:], rn1[:])
            if t == NUM_ROUTING:
                vt = itp.tile([BL, N], F32, tag="vt")
                nc.vector.tensor_mul(
                    vt[:].rearrange("b (oa o) -> b oa o", o=O),
                    s_t[:].rearrange("b (oa o) -> b oa o", o=O),
                    f[:].unsqueeze(2).broadcast_to([BL, OA, O]),
                )
                nc.sync.dma_start(out=vout_d[:], in_=vt[:])
                break

            vtb = itp.tile([BL, N], BF16, tag="vtb")
            nc.vector.tensor_mul(
                vtb[:].rearrange("b (oa o) -> b oa o", o=O),
                s_t[:].rearrange("b (oa o) -> b oa o", o=O),
                f[:].unsqueeze(2).broadcast_to([BL, OA, O]),
            )
            vr_ps = pss.tile([P, N], F32, tag="vrps")
            nc.tensor.matmul(vr_ps[:], lhsT=brep[:], rhs=vtb[:], start=True, stop=True)
            vrep = itp.tile([P, N], BF16, tag="vrep")
            # DVE copy, not scalar: an ACT Copy would force the exp-set table
            # to reload right on the critical path
            nc.vector.tensor_copy(vrep[:], vr_ps[:])

            tmp = itp.tile([P, C, N], BF16, tag="big")
            vr_b = vrep[:].unsqueeze(1).broadcast_to([P, C, N])
            nc.vector.tensor_mul(tmp[:], votes[:], vr_b[:])
            t4 = tmp[:].rearrange("p c (oa o) -> p c oa o", o=O)
            for h in (8, 4, 2):
                nc.vector.tensor_add(
                    t4[:, :, 0:h, :], t4[:, :, 0:h, :], t4[:, :, h : 2 * h, :]
                )
            H2 = C // 2
            if t == 1:
                for hs in (slice(0, H2), slice(H2, C)):
                    nc.vector.tensor_add(
                        logits[:, hs], t4[:, hs, 0, :], t4[:, hs, 1, :]
                    )
            else:
                # one more (bf16, 2x) tree level, then a single f32 accumulate
                nc.vector.tensor_add(
                    t4[:, :, 0, :], t4[:, :, 0, :], t4[:, :, 1, :]
                )
                nc.vector.tensor_add(logits[:], logits[:], t4[:, :, 0, :])

    nc.compile()
    return nc


def get_nc():
    if "nc" not in _NC_CACHE:
        _NC_CACHE["nc"] = _build_nc()
    return _NC_CACHE["nc"]


def make_in_maps(x, weights, biases):
    bf = ml_dtypes.bfloat16
    x = np.asarray(x, np.float32)
    weights = np.asarray(weights, np.float32)
    biases = np.asarray(biases, np.float32)

    # w2c[(is, h, a), cp, h2*N + (oa, o)] = w[(2cp+h)*8+is, a, o*16+oa] * (h==h2)
    w5 = (
        weights.reshape(CP, 2, IS8, A, O, OA)
        .transpose(0, 1, 2, 3, 5, 4)
        .reshape(CP, 2, IS8, A, N)
    )
    w5 = w5.transpose(0, 2, 1, 3, 4)  # [CP, is, h, a, N]
    w2c = np.zeros((CP, IS8, 2, A, 2, N), np.float32)
    for h in range(2):
        w2c[:, :, h, :, h, :] = w5[:, :, h]
    w2c = w2c.reshape(CP, P, N2).transpose(1, 0, 2).astype(bf)

    eye = np.eye(BL, dtype=np.float32)
    bsel = np.tile(eye, (IS8, 1)).astype(bf)  # bsel[p, b'] = delta(p % BL == b')
    brep = np.tile(eye, (1, IS8)).astype(bf)  # brep[b, p] = delta(b == p % BL)
    biasr = np.broadcast_to(biases.T.reshape(1, N), (BL, N)).astype(np.float32).copy()

    in_maps = []
    for k in range(NCORES):
        xc = x[k * BL : (k + 1) * BL]  # [BL, I, A]
        # xcomp[(is, h, a), cp, (is&1)*16 + b] = x[b, (2cp+h)*8+is, a]
        # (padded to 32-wide column blocks so the on-chip diagonal scatter
        #  can use 32-partition-aligned copies)
        xt = (
            xc.reshape(BL, CP, 2, IS8, A)
            .transpose(3, 2, 4, 1, 0)  # [is, h, a, cp, b]
            .reshape(IS8 // 2, 2, 2 * A, CP, BL)  # [is2, isl, (h,a), cp, b]
        )
        xcomp = np.zeros((IS8 // 2, 2, 2 * A, CP, 2, BL), np.float32)
        for isl in range(2):
            xcomp[:, isl, :, :, isl, :] = xt[:, isl]
        xcomp = xcomp.reshape(P, CP, 2 * BL).astype(bf)
        in_maps.append(
            {
                "xcomp": np.ascontiguousarray(xcomp),
                "w2c": w2c,
                "bsel": bsel,
                "brep": brep,
                "biasr": biasr,
            }
        )
    return in_maps


def assemble_out(results):
    out = np.zeros((B, 1, O, OA), np.float32)
    for k in range(NCORES):
        v = np.asarray(results[k]["vout"], np.float32)  # [BL, N], n = oa*O + o
        out[k * BL : (k + 1) * BL, 0] = v.reshape(BL, OA, O).transpose(0, 2, 1)
    return out


def kernel(x, weights, biases):
    from concourse.bass_utils import run_bass_kernel_spmd

    nc = get_nc()
    in_maps = make_in_maps(x, weights, biases)
    res = run_bass_kernel_spmd(nc, in_maps, list(range(NCORES)))
    return assemble_out(res.results)
